# revision 33
# baseline (speedup 1.0000x reference)
"""AnemllQATLinear Trainium2 kernel (8 NeuronCores, column-parallel).

y = x @ fake_quant(weight).T + bias + lora_scaling * (x @ lora_A.T) @ lora_B.T

Strategy:
  - Shard out_features (O=4096) across 8 cores (512 each). Replicate x.
  - Host prep: x -> x^T as bf16 [I, N] (shared); per-core weight shard
    transposed [I, 512] f32; scale tensors derived from scale_A@scale_B.
  - Device per core: fake-quantize the weight shard into bf16 wq^T tiles
    (closed-form uniform-LUT quantizer, magic-number rounding), then a
    K-cached tiled matmul y[N, 512] = (x^T).T @ wq^T with fused bias add.
  - LoRA is folded into the weight: W_eff = wq + lora_scaling*(lora_B@lora_A).
  - Host gathers per-core y slices -> full [4, 4096, 4096] f32.
"""
import sys
import types
from contextlib import ExitStack

import numpy as np
import ml_dtypes

import concourse.bass as bass
import concourse.mybir as mybir
import concourse.tile as tile
from concourse import bacc
from concourse.bass_utils import run_bass_kernel_spmd


P = 128
N_CORES = 8
O_FULL = 4096
O_LOC = O_FULL // N_CORES  # 512
I_DIM = 4096               # contraction dim K
B, S = 4, 4096
N_ROWS = B * S             # 16384
GS = 128                   # quant group size (== P, so one k-tile == one group)
G = I_DIM // GS            # 32 groups
EPS = 1e-8
LUT_SIZE = 16
LORA_SCALING = 2.0
MAGIC = 12582912.0         # 1.5 * 2**23: f32 round-to-nearest-int via add/sub
QSTEP = 2.0 / (LUT_SIZE - 1)
HALF_IDX = (LUT_SIZE - 1) / 2.0  # 7.5

F32 = mybir.dt.float32
BF16 = mybir.dt.bfloat16
ALU = mybir.AluOpType


def _install_ntff_hook():
    """Enable trace=True under axon: bass_utils needs antenv.axon_hooks."""
    try:
        import antenv

        if "antenv.axon_hooks" not in sys.modules:
            mod = types.ModuleType("antenv.axon_hooks")
            mod._hook = None
            mod.set_axon_ntff_profile_hook = lambda h: setattr(mod, "_hook", h)
            mod.get_axon_ntff_profile_hook = lambda: mod._hook
            sys.modules["antenv.axon_hooks"] = mod
            antenv.axon_hooks = mod
        from trn_agent_boot.trn_boot import _ntff_profile_via_ctypes

        sys.modules["antenv.axon_hooks"].set_axon_ntff_profile_hook(
            _ntff_profile_via_ctypes("/opt/axon/libaxon_pjrt.so")
        )
        import concourse.bass_utils as bass_utils

        bass_utils.upload_artifacts = lambda tmpdir: str(tmpdir)
    except Exception:
        pass


def build_nc(use_lora: bool, a_fit: float, b_fit: float):
    nc = bacc.Bacc("TRN2", target_bir_lowering=False, debug=False, num_devices=N_CORES)

    xt = nc.dram_tensor("xt", [I_DIM, N_ROWS], BF16, kind="ExternalInput")
    wt = nc.dram_tensor("wt", [I_DIM, O_LOC], F32, kind="ExternalInput")
    # scale-derived per-group tensors, transposed to [G, O_LOC]:
    #   rs = half_idx / s   (normalize factor, f32; clamp at +-half_idx)
    #   sb = s              (rescale, bf16 is plenty)
    rs = nc.dram_tensor("rs", [1, G, O_LOC], F32, kind="ExternalInput")
    sb = nc.dram_tensor("sb", [1, G, O_LOC], BF16, kind="ExternalInput")
    bias_in = nc.dram_tensor("biasv", [1, O_LOC], F32, kind="ExternalInput")
    if use_lora:
        # aw = lora-fold term (lora_scaling * (lora_B@lora_A))^T shard
        aw = nc.dram_tensor("aw", [I_DIM, O_LOC], F32, kind="ExternalInput")
    else:
        aw = None
    y = nc.dram_tensor("y", [N_ROWS, O_LOC], F32, kind="ExternalOutput")

    K_TILE = 512
    K_TILES_N = I_DIM // K_TILE  # 8
    K_SUB = K_TILE // P          # 4 groups per k-tile

    with ExitStack() as ctx:
        tc = ctx.enter_context(tile.TileContext(nc))
        constp = ctx.enter_context(tc.tile_pool(name="const", bufs=1))
        qpool = ctx.enter_context(tc.tile_pool(name="qpool", bufs=3))
        qbc = ctx.enter_context(tc.tile_pool(name="qbc", bufs=2))
        wq_pool = ctx.enter_context(tc.tile_pool(name="wq_pool", bufs=1))
        # one pair in flight (16 tiles) + prefetch window for the next pair
        kxm_pool = ctx.enter_context(tc.tile_pool(name="kxm_pool", bufs=18))

        # bias broadcast to all partitions once
        bias_bc = constp.tile([P, O_LOC], F32)
        nc.sync.dma_start(out=bias_bc[:], in_=bias_in[:].broadcast_to([P, O_LOC]))

        # ---- Phase A: fake-quantize weight shard into SBUF-resident wq^T tiles
        # (bf16, [128, K_SUB, O_LOC] per k-tile) that phase B reads directly ----
        wq_tiles = [
            wq_pool.tile([P, K_SUB, O_LOC], BF16, tag=f"wqt{k}", name=f"wqt{k}")
            for k in range(K_TILES_N)
        ]

        # prefetch the first m-tile PAIR's kxm tiles, interleaved with the
        # quantize loads on the HWDGE queues (emitted inside the loop below)
        xv = xt[:].rearrange("(po pi) f -> pi po f", pi=P)   # [128, G, N_ROWS]
        wv = wt[:].rearrange("(po pi) f -> pi po f", pi=P)   # [128, G, O_LOC]
        M_TILE = 512
        prefetched = {}

        # quantize one k-tile (4 groups = [128, 4, 512] = 2048 free elems) per
        # iteration: 4x fewer ops/DMAs, and each iteration finishes a whole
        # wq k-tile that unlocks 32 matmuls of the first m-tile pair
        for k in range(K_TILES_N):
            gsl = slice(k * K_SUB, (k + 1) * K_SUB)
            wt_t = qpool.tile([P, K_SUB, O_LOC], F32, tag="wt")
            nc.sync.dma_start(out=wt_t[:], in_=wv[:, gsl, :])
            for mt in range(2):
                t = kxm_pool.tile([P, K_SUB, M_TILE], BF16, tag="kxm",
                                  name=f"kxmp_{mt}_{k}")
                nc.sync.dma_start(
                    out=t[:],
                    in_=xv[:, gsl, mt * M_TILE:(mt + 1) * M_TILE])
                prefetched[(mt, k)] = t
            # scale broadcasts via SWDGE replicating DMAs: DMA ports don't
            # contend with DVE (GpSimd tensor work would — shared SBUF port)
            rB = qbc.tile([P, K_SUB, O_LOC], F32, tag="rB")
            nc.gpsimd.dma_start(
                out=rB[:], in_=rs[:, gsl, :].broadcast_to([P, K_SUB, O_LOC]))
            sB = qbc.tile([P, K_SUB, O_LOC], BF16, tag="sB")
            nc.gpsimd.dma_start(
                out=sB[:], in_=sb[:, gsl, :].broadcast_to([P, K_SUB, O_LOC]))
            if use_lora:
                awB = qbc.tile([P, K_SUB, O_LOC], F32, tag="awB")
                nc.gpsimd.dma_start(
                    out=awB[:],
                    in_=aw[:].rearrange("(po pi) f -> pi po f", pi=P)[:, gsl, :])

            u = qpool.tile([P, K_SUB, O_LOC], F32, tag="u")
            # u = w * (half_idx/s)
            nc.vector.tensor_tensor(out=u[:], in0=wt_t[:], in1=rB[:], op=ALU.mult)
            # t = clamp(u + half_idx, 0, 15)
            nc.vector.tensor_scalar(
                out=u[:], in0=u[:], scalar1=HALF_IDX, scalar2=0.0,
                op0=ALU.add, op1=ALU.max,
            )
            # t3 = min(t, 15) + MAGIC  -> MAGIC + round(t)
            nc.vector.tensor_scalar(
                out=u[:], in0=u[:], scalar1=float(LUT_SIZE - 1), scalar2=MAGIC,
                op0=ALU.min, op1=ALU.add,
            )
            # v = (t3 - MAGIC) * lut_slope     (= idx * b, idx in [0,15])
            nc.vector.tensor_scalar(
                out=u[:], in0=u[:], scalar1=MAGIC, scalar2=b_fit,
                op0=ALU.subtract, op1=ALU.mult,
            )
            # wq = (v + lut_intercept) * s  [+ lora fold]   (cast to bf16)
            if use_lora:
                nc.vector.scalar_tensor_tensor(
                    out=u[:], in0=u[:], scalar=a_fit, in1=sB[:],
                    op0=ALU.add, op1=ALU.mult,
                )
                nc.vector.tensor_tensor(
                    out=wq_tiles[k][:], in0=u[:], in1=awB[:], op=ALU.add,
                )
            else:
                nc.vector.scalar_tensor_tensor(
                    out=wq_tiles[k][:], in0=u[:], scalar=a_fit, in1=sB[:],
                    op0=ALU.add, op1=ALU.mult,
                )

        # ---- Phase B: y[N, O_LOC] = (x^T).T @ wq^T + bias ----
        # Custom loop: m-tiles processed in PAIRS with k OUTERMOST inside each
        # pair, so during the quantize-trailing phase the PE has 8 runnable
        # matmuls per freshly quantized group (2 m-tiles x 4 m_inner), using
        # all 8 PSUM banks. This keeps the PE fed while wq is still being
        # produced, and is equivalent afterwards.
        yv = y[:].rearrange("(po pi) f -> pi po f", pi=P)    # [128, N/128, O_LOC]
        M_TILES = N_ROWS // M_TILE  # 32
        M_SUB = M_TILE // P         # 4

        psum_pool = ctx.enter_context(
            tc.tile_pool(name="psum_pool", bufs=1, space="PSUM"))
        ypool = ctx.enter_context(tc.tile_pool(name="ypool", bufs=6))

        for pair in range(M_TILES // 2):
            kxm_t = {}
            for m01 in range(2):
                mt = pair * 2 + m01
                for k in range(K_TILES_N):
                    if (mt, k) in prefetched:
                        kxm_t[(m01, k)] = prefetched[(mt, k)]
                        continue
                    t = kxm_pool.tile(
                        [P, K_SUB, M_TILE], BF16, tag="kxm", name=f"kxm_{mt}_{k}")
                    nc.sync.dma_start(
                        out=t[:],
                        in_=xv[:, k * K_SUB:(k + 1) * K_SUB,
                               mt * M_TILE:(mt + 1) * M_TILE])
                    kxm_t[(m01, k)] = t
            ps = [
                [psum_pool.tile([P, O_LOC], F32, tag=f"ps{m01}_{j}",
                                name=f"ps{m01}_{j}_{pair}")
                 for j in range(M_SUB)]
                for m01 in range(2)
            ]
            last_pair = pair == M_TILES // 2 - 1
            if last_pair:
                # m-major so m01=0's evict+store overlaps m01=1's matmuls,
                # shrinking the kernel tail
                mm_order = [(m01, k, ki) for m01 in range(2)
                            for k in range(K_TILES_N) for ki in range(K_SUB)]
            else:
                # k-major across the pair: 8 runnable matmuls per quantized
                # group while trailing the quantizer
                mm_order = [(m01, k, ki) for k in range(K_TILES_N)
                            for ki in range(K_SUB) for m01 in range(2)]

            def evict(m01):
                mt_ = pair * 2 + m01
                for j in range(M_SUB):
                    yt = ypool.tile([P, O_LOC], F32, tag="yt", name=f"yt{mt_}{j}")
                    nc.vector.tensor_tensor(
                        out=yt[:], in0=ps[m01][j][:], in1=bias_bc[:], op=ALU.add)
                    nc.sync.dma_start(out=yv[:, mt_ * M_SUB + j, :], in_=yt[:])

            for m01, k, ki in mm_order:
                rhs = wq_tiles[k][:, ki, :]
                for j in range(M_SUB):
                    nc.tensor.matmul(
                        ps[m01][j][:],
                        kxm_t[(m01, k)][:, ki, bass.ts(j, P)],
                        rhs,
                        start=(k == 0 and ki == 0),
                        stop=(k == K_TILES_N - 1 and ki == K_SUB - 1),
                    )
                if last_pair and m01 == 0 and k == K_TILES_N - 1 and ki == K_SUB - 1:
                    evict(0)
            evict(1)
            if not last_pair:
                evict(0)

    nc.compile()
    return nc


_NC_CACHE: dict = {}


def _get_nc(use_lora: bool, a_fit: float, b_fit: float):
    key = (use_lora, a_fit, b_fit)
    if key not in _NC_CACHE:
        _NC_CACHE[key] = build_nc(use_lora, a_fit, b_fit)
    return _NC_CACHE[key]


def kernel(x, weight, bias, scale_A, scale_B, lut, lora_A, lora_B, **_):
    _install_ntff_hook()

    x = np.asarray(x, dtype=np.float32)
    weight = np.asarray(weight, dtype=np.float32)
    bias = np.asarray(bias, dtype=np.float32)
    scale_A = np.asarray(scale_A, dtype=np.float32)
    scale_B = np.asarray(scale_B, dtype=np.float32)
    lut = np.asarray(lut, dtype=np.float32)
    lora_A = np.asarray(lora_A, dtype=np.float32)
    lora_B = np.asarray(lora_B, dtype=np.float32)

    # ---- host prep ----
    s_full = np.maximum(scale_A @ scale_B, EPS)  # [O, G]

    # affine fit of the LUT: lut[k] ~= a + b*k (exact for linspace)
    a_fit = float(lut[0])
    b_fit = float(lut[-1] - lut[0]) / (LUT_SIZE - 1)
    idx = np.arange(LUT_SIZE, dtype=np.float32)
    affine_ok = np.max(np.abs(lut - (a_fit + b_fit * idx))) <= 1e-6 * max(
        1.0, np.max(np.abs(lut))
    )

    wl = None
    use_lora = bool(np.any(lora_B != 0.0)) or not affine_ok
    if use_lora:
        wl = (LORA_SCALING * (lora_B @ lora_A)).astype(np.float32)  # [O, I]

    if not affine_ok:
        # general LUT fallback: quantize on host, ship wq via the lora path
        grouped = weight.reshape(O_FULL, G, GS)
        norm = np.clip(grouped / s_full[:, :, None], -1.0, 1.0)
        qidx = np.clip(
            np.round((norm + 1.0) / QSTEP).astype(np.int32), 0, LUT_SIZE - 1
        )
        wq_host = (lut[qidx] * s_full[:, :, None]).reshape(O_FULL, I_DIM)
        wl = wl + wq_host if wl is not None else wq_host
        # zero out the device quantizer: u=0, (0+a)*0 = 0, + wl = wq_host
        rs_full = np.zeros_like(s_full)
        sb_full = np.zeros_like(s_full)
        a_dev, b_dev = 0.0, 1.0
    else:
        rs_full = (HALF_IDX / s_full).astype(np.float32)  # [O, G]
        sb_full = s_full
        a_dev, b_dev = float(a_fit), float(b_fit)

    x2 = x.reshape(N_ROWS, I_DIM)
    xt_bf16 = np.ascontiguousarray(x2.astype(ml_dtypes.bfloat16).T)  # [I, N]

    in_maps = []
    for c in range(N_CORES):
        sl = slice(c * O_LOC, (c + 1) * O_LOC)
        m = {
            "xt": xt_bf16,
            "wt": np.ascontiguousarray(weight[sl].T),          # [I, O_LOC]
            "rs": np.ascontiguousarray(rs_full[sl].T).reshape(1, G, O_LOC),
            "sb": np.ascontiguousarray(sb_full[sl].T).astype(
                ml_dtypes.bfloat16).reshape(1, G, O_LOC),
            "biasv": bias[sl].reshape(1, O_LOC).copy(),
        }
        if use_lora:
            m["aw"] = np.ascontiguousarray(wl[sl].T)           # [I, O_LOC]
        in_maps.append(m)

    nc = _get_nc(use_lora, a_dev, b_dev)
    res = run_bass_kernel_spmd(
        nc, in_maps, core_ids=list(range(N_CORES)), trace=False
    )
    global LAST_RESULT
    LAST_RESULT = res

    y = np.concatenate([res.results[c]["y"] for c in range(N_CORES)], axis=1)
    return np.ascontiguousarray(y.reshape(B, S, O_FULL).astype(np.float32))


if __name__ == "__main__":
    rng = np.random.default_rng(0)
    x = rng.standard_normal((B, S, I_DIM), dtype=np.float32)
    weight = (rng.standard_normal((O_FULL, I_DIM), dtype=np.float32) * 0.02)
    bias = rng.uniform(-0.015, 0.015, O_FULL).astype(np.float32)
    sf = np.maximum(np.abs(weight.reshape(O_FULL, G, GS)).max(axis=2), EPS)
    u, s, vh = np.linalg.svd(sf, full_matrices=False)
    scale_A = (u[:, :4] * s[:4]).astype(np.float32)
    scale_B = vh[:4, :].astype(np.float32)
    lut = np.linspace(-1, 1, LUT_SIZE, dtype=np.float32)
    lora_A = rng.standard_normal((16, I_DIM), dtype=np.float32) * 0.02
    lora_B = np.zeros((O_FULL, 16), dtype=np.float32)
    y = kernel(x=x, weight=weight, bias=bias, scale_A=scale_A, scale_B=scale_B,
               lut=lut, lora_A=lora_A, lora_B=lora_B)
    print("kernel output:", y.shape, y.dtype)


# revision 37
# speedup vs baseline: 1.1808x; 1.1808x over previous
"""AnemllQATLinear Trainium2 kernel (8 NeuronCores, column-parallel).

y = x @ fake_quant(weight).T + bias + lora_scaling * (x @ lora_A.T) @ lora_B.T

Strategy:
  - Shard out_features (O=4096) across 8 cores (512 each). Replicate x.
  - Host prep: x -> x^T as bf16 [I, N] (shared); per-core weight shard
    transposed [I, 512] f32; scale tensors derived from scale_A@scale_B.
  - Device per core: fake-quantize the weight shard into bf16 wq^T tiles
    (closed-form uniform-LUT quantizer, magic-number rounding), then a
    K-cached tiled matmul y[N, 512] = (x^T).T @ wq^T with fused bias add.
  - LoRA is folded into the weight: W_eff = wq + lora_scaling*(lora_B@lora_A).
  - Host gathers per-core y slices -> full [4, 4096, 4096] f32.
"""
import sys
import types
from contextlib import ExitStack

import numpy as np
import ml_dtypes

import concourse.bass as bass
import concourse.mybir as mybir
import concourse.tile as tile
from concourse import bacc
from concourse.bass_utils import run_bass_kernel_spmd


P = 128
N_CORES = 8
O_FULL = 4096
O_LOC = O_FULL // N_CORES  # 512
I_DIM = 4096               # contraction dim K
B, S = 4, 4096
N_ROWS = B * S             # 16384
GS = 128                   # quant group size (== P, so one k-tile == one group)
G = I_DIM // GS            # 32 groups
EPS = 1e-8
LUT_SIZE = 16
LORA_SCALING = 2.0
MAGIC = 12582912.0         # 1.5 * 2**23: f32 round-to-nearest-int via add/sub
QSTEP = 2.0 / (LUT_SIZE - 1)
HALF_IDX = (LUT_SIZE - 1) / 2.0  # 7.5

F32 = mybir.dt.float32
BF16 = mybir.dt.bfloat16
ALU = mybir.AluOpType


def _install_ntff_hook():
    """Enable trace=True under axon: bass_utils needs antenv.axon_hooks."""
    try:
        import antenv

        if "antenv.axon_hooks" not in sys.modules:
            mod = types.ModuleType("antenv.axon_hooks")
            mod._hook = None
            mod.set_axon_ntff_profile_hook = lambda h: setattr(mod, "_hook", h)
            mod.get_axon_ntff_profile_hook = lambda: mod._hook
            sys.modules["antenv.axon_hooks"] = mod
            antenv.axon_hooks = mod
        from trn_agent_boot.trn_boot import _ntff_profile_via_ctypes

        sys.modules["antenv.axon_hooks"].set_axon_ntff_profile_hook(
            _ntff_profile_via_ctypes("/opt/axon/libaxon_pjrt.so")
        )
        import concourse.bass_utils as bass_utils

        bass_utils.upload_artifacts = lambda tmpdir: str(tmpdir)
    except Exception:
        pass


def build_nc(use_lora: bool, a_fit: float, b_fit: float):
    nc = bacc.Bacc("TRN2", target_bir_lowering=False, debug=False, num_devices=N_CORES)

    xt = nc.dram_tensor("xt", [I_DIM, N_ROWS], BF16, kind="ExternalInput")
    wt = nc.dram_tensor("wt", [I_DIM, O_LOC], F32, kind="ExternalInput")
    # scale-derived per-group tensors, transposed to [G, O_LOC]:
    #   rs = half_idx / s   (normalize factor, f32; clamp at +-half_idx)
    #   sb = s              (rescale, bf16 is plenty)
    rs = nc.dram_tensor("rs", [1, G, O_LOC], F32, kind="ExternalInput")
    sb = nc.dram_tensor("sb", [1, G, O_LOC], BF16, kind="ExternalInput")
    bias_in = nc.dram_tensor("biasv", [1, O_LOC], F32, kind="ExternalInput")
    if use_lora:
        # aw = lora-fold term (lora_scaling * (lora_B@lora_A))^T shard
        aw = nc.dram_tensor("aw", [I_DIM, O_LOC], F32, kind="ExternalInput")
    else:
        aw = None
    y = nc.dram_tensor("y", [N_ROWS, O_LOC], F32, kind="ExternalOutput")

    K_TILE = 512
    K_TILES_N = I_DIM // K_TILE  # 8
    K_SUB = K_TILE // P          # 4 groups per k-tile

    with ExitStack() as ctx:
        tc = ctx.enter_context(tile.TileContext(nc))
        constp = ctx.enter_context(tc.tile_pool(name="const", bufs=1))
        qpool = ctx.enter_context(tc.tile_pool(name="qpool", bufs=3))
        qbc = ctx.enter_context(tc.tile_pool(name="qbc", bufs=2))
        wq_pool = ctx.enter_context(tc.tile_pool(name="wq_pool", bufs=1))
        # one pair in flight (16 tiles) + prefetch window for the next pair
        kxm_pool = ctx.enter_context(tc.tile_pool(name="kxm_pool", bufs=18))

        # bias broadcast to all partitions once
        bias_bc = constp.tile([P, O_LOC], F32)
        nc.sync.dma_start(out=bias_bc[:], in_=bias_in[:].broadcast_to([P, O_LOC]))

        # ---- Phase A: fake-quantize weight shard into SBUF-resident wq^T tiles
        # (bf16, [128, K_SUB, O_LOC] per k-tile) that phase B reads directly ----
        wq_tiles = [
            wq_pool.tile([P, K_SUB, O_LOC], BF16, tag=f"wqt{k}", name=f"wqt{k}")
            for k in range(K_TILES_N)
        ]

        # prefetch the first m-tile PAIR's kxm tiles, interleaved with the
        # quantize loads on the HWDGE queues (emitted inside the loop below)
        xv = xt[:].rearrange("(po pi) f -> pi po f", pi=P)   # [128, G, N_ROWS]
        wv = wt[:].rearrange("(po pi) f -> pi po f", pi=P)   # [128, G, O_LOC]
        M_TILE = 512
        prefetched = {}

        # quantize chunks: mostly one whole k-tile (4 groups = [128, 4, 512] =
        # 2048 free elems) per iteration — 4x fewer ops/DMAs — but the first
        # k-tile is split fine-grained so the PE's first matmuls start early
        chunks = [(0, 0, 1), (0, 1, 1), (0, 2, 2)] + [
            (k, 0, K_SUB) for k in range(1, K_TILES_N)
        ]
        for ci, (k, g0_, gn) in enumerate(chunks):
            gsl = slice(k * K_SUB + g0_, k * K_SUB + g0_ + gn)
            csl = slice(g0_, g0_ + gn)
            wt_t = qpool.tile([P, K_SUB, O_LOC], F32, tag="wt", name="wt_t")[:, :gn, :]
            nc.sync.dma_start(out=wt_t[:], in_=wv[:, gsl, :])
            for mt in range(2):
                if (mt, k) in prefetched:
                    continue
                t = kxm_pool.tile([P, K_SUB, M_TILE], BF16, tag="kxm",
                                  name=f"kxmp_{mt}_{k}")
                nc.sync.dma_start(
                    out=t[:],
                    in_=xv[:, k * K_SUB:(k + 1) * K_SUB,
                           mt * M_TILE:(mt + 1) * M_TILE])
                prefetched[(mt, k)] = t
            # scale broadcasts via SWDGE replicating DMAs: DMA ports don't
            # contend with DVE (GpSimd tensor work would — shared SBUF port)
            rB = qbc.tile([P, K_SUB, O_LOC], F32, tag="rB", name="rB")[:, :gn, :]
            nc.gpsimd.dma_start(
                out=rB[:], in_=rs[:, gsl, :].broadcast_to([P, gn, O_LOC]))
            sB = qbc.tile([P, K_SUB, O_LOC], BF16, tag="sB", name="sB")[:, :gn, :]
            nc.gpsimd.dma_start(
                out=sB[:], in_=sb[:, gsl, :].broadcast_to([P, gn, O_LOC]))
            if use_lora:
                awB = qbc.tile([P, K_SUB, O_LOC], F32, tag="awB", name="awB")[:, :gn, :]
                nc.gpsimd.dma_start(
                    out=awB[:],
                    in_=aw[:].rearrange("(po pi) f -> pi po f", pi=P)[:, gsl, :])

            u = qpool.tile([P, K_SUB, O_LOC], F32, tag="u", name="u")[:, :gn, :]
            # u = w * (half_idx/s)
            nc.vector.tensor_tensor(out=u[:], in0=wt_t[:], in1=rB[:], op=ALU.mult)
            # t = clamp(u + half_idx, 0, 15)
            nc.vector.tensor_scalar(
                out=u[:], in0=u[:], scalar1=HALF_IDX, scalar2=0.0,
                op0=ALU.add, op1=ALU.max,
            )
            # t3 = min(t, 15) + MAGIC  -> MAGIC + round(t)
            nc.vector.tensor_scalar(
                out=u[:], in0=u[:], scalar1=float(LUT_SIZE - 1), scalar2=MAGIC,
                op0=ALU.min, op1=ALU.add,
            )
            # v = (t3 - MAGIC) * lut_slope     (= idx * b, idx in [0,15])
            nc.vector.tensor_scalar(
                out=u[:], in0=u[:], scalar1=MAGIC, scalar2=b_fit,
                op0=ALU.subtract, op1=ALU.mult,
            )
            # wq = (v + lut_intercept) * s  [+ lora fold]   (cast to bf16)
            if use_lora:
                nc.vector.scalar_tensor_tensor(
                    out=u[:], in0=u[:], scalar=a_fit, in1=sB[:],
                    op0=ALU.add, op1=ALU.mult,
                )
                nc.vector.tensor_tensor(
                    out=wq_tiles[k][:, csl, :], in0=u[:], in1=awB[:], op=ALU.add,
                )
            else:
                nc.vector.scalar_tensor_tensor(
                    out=wq_tiles[k][:, csl, :], in0=u[:], scalar=a_fit, in1=sB[:],
                    op0=ALU.add, op1=ALU.mult,
                )

        # ---- Phase B: y[N, O_LOC] = (x^T).T @ wq^T + bias ----
        # Custom loop: m-tiles processed in PAIRS with k OUTERMOST inside each
        # pair, so during the quantize-trailing phase the PE has 8 runnable
        # matmuls per freshly quantized group (2 m-tiles x 4 m_inner), using
        # all 8 PSUM banks. This keeps the PE fed while wq is still being
        # produced, and is equivalent afterwards.
        yv = y[:].rearrange("(po pi) f -> pi po f", pi=P)    # [128, N/128, O_LOC]
        M_TILES = N_ROWS // M_TILE  # 32
        M_SUB = M_TILE // P         # 4

        psum_pool = ctx.enter_context(
            tc.tile_pool(name="psum_pool", bufs=1, space="PSUM"))
        ypool = ctx.enter_context(tc.tile_pool(name="ypool", bufs=6))

        for pair in range(M_TILES // 2):
            kxm_t = {}
            for m01 in range(2):
                mt = pair * 2 + m01
                for k in range(K_TILES_N):
                    if (mt, k) in prefetched:
                        kxm_t[(m01, k)] = prefetched[(mt, k)]
                        continue
                    t = kxm_pool.tile(
                        [P, K_SUB, M_TILE], BF16, tag="kxm", name=f"kxm_{mt}_{k}")
                    nc.sync.dma_start(
                        out=t[:],
                        in_=xv[:, k * K_SUB:(k + 1) * K_SUB,
                               mt * M_TILE:(mt + 1) * M_TILE])
                    kxm_t[(m01, k)] = t
            ps = [
                [psum_pool.tile([P, O_LOC], F32, tag=f"ps{m01}_{j}",
                                name=f"ps{m01}_{j}_{pair}")
                 for j in range(M_SUB)]
                for m01 in range(2)
            ]
            last_pair = pair == M_TILES // 2 - 1
            if last_pair:
                # m-major so m01=0's evict+store overlaps m01=1's matmuls,
                # shrinking the kernel tail
                mm_order = [(m01, k, ki) for m01 in range(2)
                            for k in range(K_TILES_N) for ki in range(K_SUB)]
            else:
                # k-major across the pair: 8 runnable matmuls per quantized
                # group while trailing the quantizer
                mm_order = [(m01, k, ki) for k in range(K_TILES_N)
                            for ki in range(K_SUB) for m01 in range(2)]

            def evict(m01):
                mt_ = pair * 2 + m01
                for j in range(M_SUB):
                    yt = ypool.tile([P, O_LOC], F32, tag="yt", name=f"yt{mt_}{j}")
                    nc.vector.tensor_tensor(
                        out=yt[:], in0=ps[m01][j][:], in1=bias_bc[:], op=ALU.add)
                    nc.sync.dma_start(out=yv[:, mt_ * M_SUB + j, :], in_=yt[:])

            for m01, k, ki in mm_order:
                rhs = wq_tiles[k][:, ki, :]
                for j in range(M_SUB):
                    nc.tensor.matmul(
                        ps[m01][j][:],
                        kxm_t[(m01, k)][:, ki, bass.ts(j, P)],
                        rhs,
                        start=(k == 0 and ki == 0),
                        stop=(k == K_TILES_N - 1 and ki == K_SUB - 1),
                    )
                if last_pair and m01 == 0 and k == K_TILES_N - 1 and ki == K_SUB - 1:
                    evict(0)
            evict(1)
            if not last_pair:
                evict(0)

    nc.compile()
    return nc


_NC_CACHE: dict = {}


def _get_nc(use_lora: bool, a_fit: float, b_fit: float):
    key = (use_lora, a_fit, b_fit)
    if key not in _NC_CACHE:
        _NC_CACHE[key] = build_nc(use_lora, a_fit, b_fit)
    return _NC_CACHE[key]


def kernel(x, weight, bias, scale_A, scale_B, lut, lora_A, lora_B, **_):
    _install_ntff_hook()

    x = np.asarray(x, dtype=np.float32)
    weight = np.asarray(weight, dtype=np.float32)
    bias = np.asarray(bias, dtype=np.float32)
    scale_A = np.asarray(scale_A, dtype=np.float32)
    scale_B = np.asarray(scale_B, dtype=np.float32)
    lut = np.asarray(lut, dtype=np.float32)
    lora_A = np.asarray(lora_A, dtype=np.float32)
    lora_B = np.asarray(lora_B, dtype=np.float32)

    # ---- host prep ----
    s_full = np.maximum(scale_A @ scale_B, EPS)  # [O, G]

    # affine fit of the LUT: lut[k] ~= a + b*k (exact for linspace)
    a_fit = float(lut[0])
    b_fit = float(lut[-1] - lut[0]) / (LUT_SIZE - 1)
    idx = np.arange(LUT_SIZE, dtype=np.float32)
    affine_ok = np.max(np.abs(lut - (a_fit + b_fit * idx))) <= 1e-6 * max(
        1.0, np.max(np.abs(lut))
    )

    wl = None
    use_lora = bool(np.any(lora_B != 0.0)) or not affine_ok
    if use_lora:
        wl = (LORA_SCALING * (lora_B @ lora_A)).astype(np.float32)  # [O, I]

    if not affine_ok:
        # general LUT fallback: quantize on host, ship wq via the lora path
        grouped = weight.reshape(O_FULL, G, GS)
        norm = np.clip(grouped / s_full[:, :, None], -1.0, 1.0)
        qidx = np.clip(
            np.round((norm + 1.0) / QSTEP).astype(np.int32), 0, LUT_SIZE - 1
        )
        wq_host = (lut[qidx] * s_full[:, :, None]).reshape(O_FULL, I_DIM)
        wl = wl + wq_host if wl is not None else wq_host
        # zero out the device quantizer: u=0, (0+a)*0 = 0, + wl = wq_host
        rs_full = np.zeros_like(s_full)
        sb_full = np.zeros_like(s_full)
        a_dev, b_dev = 0.0, 1.0
    else:
        rs_full = (HALF_IDX / s_full).astype(np.float32)  # [O, G]
        sb_full = s_full
        a_dev, b_dev = float(a_fit), float(b_fit)

    x2 = x.reshape(N_ROWS, I_DIM)
    xt_bf16 = np.ascontiguousarray(x2.astype(ml_dtypes.bfloat16).T)  # [I, N]

    in_maps = []
    for c in range(N_CORES):
        sl = slice(c * O_LOC, (c + 1) * O_LOC)
        m = {
            "xt": xt_bf16,
            "wt": np.ascontiguousarray(weight[sl].T),          # [I, O_LOC]
            "rs": np.ascontiguousarray(rs_full[sl].T).reshape(1, G, O_LOC),
            "sb": np.ascontiguousarray(sb_full[sl].T).astype(
                ml_dtypes.bfloat16).reshape(1, G, O_LOC),
            "biasv": bias[sl].reshape(1, O_LOC).copy(),
        }
        if use_lora:
            m["aw"] = np.ascontiguousarray(wl[sl].T)           # [I, O_LOC]
        in_maps.append(m)

    nc = _get_nc(use_lora, a_dev, b_dev)
    res = run_bass_kernel_spmd(
        nc, in_maps, core_ids=list(range(N_CORES)), trace=False
    )
    global LAST_RESULT
    LAST_RESULT = res

    y = np.concatenate([res.results[c]["y"] for c in range(N_CORES)], axis=1)
    return np.ascontiguousarray(y.reshape(B, S, O_FULL).astype(np.float32))


if __name__ == "__main__":
    rng = np.random.default_rng(0)
    x = rng.standard_normal((B, S, I_DIM), dtype=np.float32)
    weight = (rng.standard_normal((O_FULL, I_DIM), dtype=np.float32) * 0.02)
    bias = rng.uniform(-0.015, 0.015, O_FULL).astype(np.float32)
    sf = np.maximum(np.abs(weight.reshape(O_FULL, G, GS)).max(axis=2), EPS)
    u, s, vh = np.linalg.svd(sf, full_matrices=False)
    scale_A = (u[:, :4] * s[:4]).astype(np.float32)
    scale_B = vh[:4, :].astype(np.float32)
    lut = np.linspace(-1, 1, LUT_SIZE, dtype=np.float32)
    lora_A = rng.standard_normal((16, I_DIM), dtype=np.float32) * 0.02
    lora_B = np.zeros((O_FULL, 16), dtype=np.float32)
    y = kernel(x=x, weight=weight, bias=bias, scale_A=scale_A, scale_B=scale_B,
               lut=lut, lora_A=lora_A, lora_B=lora_B)
    print("kernel output:", y.shape, y.dtype)
